# revision 3
# baseline (speedup 1.0000x reference)
"""CAM (channel attention module) Trainium2 kernel.

Computes, for x: [B, h, w, z, C] (B=4, h=w=z=48, C=128), gamma: [1]:
    a    = x.reshape(B, N, C)            # N = 110592
    aTa  = einsum('bnc,bnd->bcd', a, a)  # [B, 128, 128] channel Gram
    s    = softmax(aTa, axis=-1)
    aaTa = einsum('bnc,bcd->bnd', a, s)
    out  = gamma * aaTa + x

Sharding: 8 cores = (batch b, half hh). Each core computes the full Gram of
its batch redundantly from an fp8 copy (no collective needed; the softmax
logits have a ~1e5 diagonal margin so fp8 Gram precision is ample), then
projects its own half of the voxels in fp32 (float32r PE mode, full rate at
free-dim 512) and applies gamma*y + x fused on the vector engine.

Host-side layouts (prepared in kernel() below):
  xg  fp8e4m3 [128, 110592]  xg[p, k*128+c] = x[b, k*128+p, c]   (Gram operand)
  xt  fp32    [128, 55296]   xt[c, n]       = x[b, hh*NH+n, c]   (proj operand)
  yt  fp32    [128, 55296]   yt[d, n]       = out[b, hh*NH+n, d] (output, transposed)
"""

import os
import sys
import types
import contextlib

import numpy as np
import ml_dtypes

import concourse.bass as bass
import concourse.mybir as mybir
import concourse.tile as tile
from concourse import bacc
from concourse.bass_utils import run_bass_kernel_spmd

B, C = 4, 128
NFULL = 48 * 48 * 48          # 110592 voxels per batch
NH = NFULL // 2               # 55296 voxels per core
CH_A = 8192                   # fp8 gram-chunk cols (64 subtiles of 128)
CH_B = 4096                   # fp32 proj-chunk cols (8 matmuls of 512)

LAST_EXEC_NS = None
LAST_RESULTS = None


def _install_ntff_hook():
    """The image's antenv lacks axon_hooks; recreate boot step 6 so
    run_bass_kernel_spmd(trace=True) can capture NTFF profiles."""
    if "antenv.axon_hooks" in sys.modules:
        return True
    try:
        mod = types.ModuleType("antenv.axon_hooks")
        mod._hook = None
        mod.set_axon_ntff_profile_hook = lambda h: setattr(mod, "_hook", h)
        mod.get_axon_ntff_profile_hook = lambda: mod._hook
        sys.modules["antenv.axon_hooks"] = mod
        from trn_agent_boot.trn_boot import _ntff_profile_via_ctypes

        hook = _ntff_profile_via_ctypes("/opt/axon/libaxon_pjrt.so")
        if hook is None:
            del sys.modules["antenv.axon_hooks"]
            return False
        mod.set_axon_ntff_profile_hook(hook)
        return True
    except Exception:
        sys.modules.pop("antenv.axon_hooks", None)
        return False


def _build(gamma: float):
    f32 = mybir.dt.float32
    f32r = mybir.dt.float32r
    f8 = mybir.dt.float8e4

    nc = bacc.Bacc("TRN2", target_bir_lowering=False, debug=False, num_devices=8)
    xg_d = nc.dram_tensor("xg", [128, NFULL], f8, kind="ExternalInput")
    xt_d = nc.dram_tensor("xt", [128, NH], f32, kind="ExternalInput")
    yt_d = nc.dram_tensor("yt", [128, NH], f32, kind="ExternalOutput")

    with tile.TileContext(nc) as tc:
        with (
            tc.tile_pool(name="pa", bufs=3) as pa,
            tc.tile_pool(name="pb", bufs=6) as pb,
            tc.tile_pool(name="po", bufs=3) as po,
            tc.tile_pool(name="ps", bufs=1) as ps,
            tc.tile_pool(name="pp", bufs=1, space="PSUM") as pp,
            tc.tile_pool(name="py", bufs=4, space="PSUM") as py,
        ):
            # ---- phase A: Gram accumulation over all 864 voxel blocks ----
            gram = pp.tile([128, 128], f32, tag="gram")
            n_mm = NFULL // 128
            mm = 0
            for c0 in range(0, NFULL, CH_A):
                csz = min(CH_A, NFULL - c0)
                g = pa.tile([128, csz], f8, tag="xg")
                nc.sync.dma_start(g[:], xg_d[:, c0 : c0 + csz])
                for j in range(csz // 128):
                    nc.tensor.matmul(
                        gram[:],
                        g[:, j * 128 : (j + 1) * 128],
                        g[:, j * 128 : (j + 1) * 128],
                        start=(mm == 0),
                        stop=(mm == n_mm - 1),
                    )
                    mm += 1

            # ---- softmax over the free axis of gram [c, d] ----
            neg_mx = ps.tile([128, 1], f32, tag="mx")
            nc.vector.reduce_max(
                neg_mx[:], gram[:], axis=mybir.AxisListType.X, negate=True
            )
            shifted = ps.tile([128, 128], f32, tag="shifted")
            # shifted = max(gram - rowmax, -85)  (clamp so exp underflows cleanly)
            nc.vector.tensor_scalar(
                shifted[:],
                gram[:],
                neg_mx[:, 0:1],
                -85.0,
                op0=mybir.AluOpType.add,
                op1=mybir.AluOpType.max,
            )
            pexp = ps.tile([128, 128], f32, tag="pexp")
            sums = ps.tile([128, 1], f32, tag="sums")
            nc.scalar.activation(
                pexp[:],
                shifted[:],
                mybir.ActivationFunctionType.Exp,
                accum_out=sums[:, 0:1],
            )
            rs = ps.tile([128, 1], f32, tag="rs")
            nc.vector.reciprocal(rs[:], sums[:])
            s_sb = ps.tile([128, 128], f32, tag="s")
            nc.vector.tensor_scalar_mul(s_sb[:], pexp[:], rs[:, 0:1])

            # ---- phase B: y^T = s^T @ x^T, out = gamma*y + x, streamed ----
            for c0 in range(0, NH, CH_B):
                csz = min(CH_B, NH - c0)
                cx = pb.tile([128, csz], f32, tag="xt")
                nc.sync.dma_start(cx[:], xt_d[:, c0 : c0 + csz])
                o = po.tile([128, csz], f32, tag="out")
                for j in range(csz // 512):
                    yp = py.tile([128, 512], f32, tag="yp")
                    sl = slice(j * 512, (j + 1) * 512)
                    nc.tensor.matmul(
                        yp[:], s_sb[:], cx[:, sl], start=True, stop=True
                    )
                    nc.vector.scalar_tensor_tensor(
                        o[:, sl],
                        yp[:],
                        gamma,
                        cx[:, sl],
                        op0=mybir.AluOpType.mult,
                        op1=mybir.AluOpType.add,
                    )
                nc.scalar.dma_start(yt_d[:, c0 : c0 + csz], o[:])

    nc.compile()
    return nc


def kernel(x, gamma):
    global LAST_EXEC_NS, LAST_RESULTS
    x = np.asarray(x, dtype=np.float32)
    gamma_f = float(np.asarray(gamma).reshape(-1)[0])
    Bx, hx, wx, zx, Cx = x.shape
    N = hx * wx * zx
    xf = np.ascontiguousarray(x.reshape(Bx, N, Cx))

    nc = _build(gamma_f)

    # per-batch fp8 gram operand, voxels-on-partitions layout
    xgs = []
    for b in range(Bx):
        xg = (
            xf[b]
            .reshape(N // 128, 128, Cx)
            .transpose(1, 0, 2)
            .reshape(128, N * Cx // 128)
        )
        xgs.append(np.ascontiguousarray(xg.astype(ml_dtypes.float8_e4m3)))

    in_maps = []
    for core in range(8):
        b, hh = core // 2, core % 2
        xt = np.ascontiguousarray(xf[b, hh * NH : (hh + 1) * NH].T)
        in_maps.append({"xg": xgs[b], "xt": xt})

    want_trace = os.environ.get("CAM_TRACE", "1") == "1" and _install_ntff_hook()
    res = None
    if want_trace:
        import concourse.bass_utils as bass_utils

        orig_upload = bass_utils.upload_artifacts
        bass_utils.upload_artifacts = lambda d: d  # no S3 in this container
        try:
            res = run_bass_kernel_spmd(
                nc,
                in_maps,
                core_ids=list(range(8)),
                trace=True,
                trace_cores=[0],
            )
            LAST_EXEC_NS = res.exec_time_ns
            if res.exec_time_ns is not None:
                print(f"HW exec time: {res.exec_time_ns} ns")
        except Exception as e:
            print(f"traced run failed ({e!r}); rerunning without trace")
            res = None
        finally:
            bass_utils.upload_artifacts = orig_upload
    if res is None:
        res = run_bass_kernel_spmd(nc, in_maps, core_ids=list(range(8)))
        LAST_EXEC_NS = res.exec_time_ns
    LAST_RESULTS = res

    out = np.empty((Bx, N, Cx), dtype=np.float32)
    for core in range(8):
        b, hh = core // 2, core % 2
        out[b, hh * NH : (hh + 1) * NH] = res.results[core]["yt"].T
    return out.reshape(Bx, hx, wx, zx, Cx)
